# revision 6
# baseline (speedup 1.0000x reference)
"""Trainium2 Bass kernel for a 2-layer bidirectional GRU + linear head.

Problem: nn_BidirectionalGRU (T=256, B=128, NIN=256, H=256, NOUT=96).

Strategy (8 NeuronCores, data-parallel over batch, 16 rows/core):
  - Gate-major layout: feature dims on SBUF partitions, (time*batch) on the
    free dim. gi and h are stored t-major so every per-step AP is contiguous
    (DVE 2x mode) and scan injection is one 64-column identity matmul.
  - The z-gate rows of all weights/biases are negated on the host so sigmoid
    yields z' = 1-z; then h' = z'*n + (h - z'*h) and the two products
    a = z'*h, c = h - a are computed while tanh runs, leaving only two DVE
    ops after tanh on the critical path.
  - fwd/bwd scans are locked in anti-phase via explicit cross-dependencies
    (each dir's w-matmuls wait on the other dir's sigmoid) so the gate chain
    of one dir overlaps the PE phase of the other.
  - Inproj blocks (N=256) are emitted just-in-time between scan steps.
"""

import functools
import sys

import numpy as np

sys.path.insert(0, "/opt/trn_rl_repo")

import ml_dtypes  # noqa: E402
import concourse.bass as bass  # noqa: E402
import concourse.tile as tile  # noqa: E402
from concourse import bacc, mybir  # noqa: E402
from concourse.tile_rust import add_dep_helper  # noqa: E402

T, B, NIN, H, NOUT = 256, 128, 256, 256, 96
NCORES = 8
BL = B // NCORES          # 16 batch rows per core
G3 = 3 * H                # 768 gate rows
NM = G3 // 128            # 6 gate-row chunks
AF = mybir.ActivationFunctionType
OP = mybir.AluOpType
BF16, F32 = mybir.dt.bfloat16, mybir.dt.float32
NCH = 256                 # inproj streaming chunk (columns)
NT = NCH // BL            # t-steps per inproj block (16)

DIRS = ("f", "b")


def build_bass(t_steps=T):
    """Build the per-core Bass program (identical on all cores)."""
    tb = t_steps * BL
    nb = tb // NCH
    nc = bacc.Bacc(None, target_bir_lowering=False, debug=False)

    xT = nc.declare_dram_parameter("xT", [2, 128, tb], BF16, isOutput=False)
    ident = nc.declare_dram_parameter("ident", [128, 128], BF16, isOutput=False)
    wih, whh, bgi, bhn = {}, {}, {}, {}
    for l in (0, 1):
        kin = 2 if l == 0 else 4
        for d in DIRS:
            wih[(l, d)] = nc.declare_dram_parameter(
                f"wih{l}{d}", [kin, 128, G3], BF16, isOutput=False)
            whh[(l, d)] = nc.declare_dram_parameter(
                f"whh{l}{d}", [2, 128, G3], BF16, isOutput=False)
            bgi[(l, d)] = nc.declare_dram_parameter(
                f"bgi{l}{d}", [128, NM], F32, isOutput=False)
            bhn[(l, d)] = nc.declare_dram_parameter(
                f"bhn{l}{d}", [128, 2, BL], BF16, isOutput=False)
    wemb = nc.declare_dram_parameter("wemb", [4, 128, NOUT], BF16, isOutput=False)
    bemb = nc.declare_dram_parameter("bemb", [NOUT, 1], F32, isOutput=False)
    outT = nc.declare_dram_parameter("outT", [NOUT, tb], F32, isOutput=True)

    with tile.TileContext(nc) as tc:
        from contextlib import ExitStack
        with ExitStack() as ctx:
            consts = ctx.enter_context(tc.tile_pool(name="consts", bufs=1))
            hpool = ctx.enter_context(tc.tile_pool(name="hstate", bufs=1))
            gipool = ctx.enter_context(tc.tile_pool(name="gi", bufs=1))
            pspool = ctx.enter_context(tc.tile_pool(name="scanps", bufs=3, space="PSUM"))
            ippool = ctx.enter_context(tc.tile_pool(name="ips", bufs=2, space="PSUM"))
            work = ctx.enter_context(tc.tile_pool(name="work", bufs=4))

            # ---- load constants ----
            sb_x = consts.tile([128, 2, tb], BF16, name="sb_x")
            for k in range(2):
                nc.sync.dma_start(out=sb_x[:, k, :], in_=xT[k])
            sb_wih, sb_whh, sb_bgi, sb_bhn = {}, {}, {}, {}
            for l in (0, 1):
                kin = 2 if l == 0 else 4
                for d in DIRS:
                    t_ih = consts.tile([128, kin, G3], BF16, name=f"sb_wih{l}{d}")
                    for k in range(kin):
                        nc.sync.dma_start(out=t_ih[:, k, :], in_=wih[(l, d)][k])
                    sb_wih[(l, d)] = t_ih
                    t_hh = consts.tile([128, 2, G3], BF16, name=f"sb_whh{l}{d}")
                    for k in range(2):
                        nc.sync.dma_start(out=t_hh[:, k, :], in_=whh[(l, d)][k])
                    sb_whh[(l, d)] = t_hh
                    t_bg = consts.tile([128, NM], F32, name=f"sb_bgi{l}{d}")
                    nc.sync.dma_start(out=t_bg, in_=bgi[(l, d)][:])
                    sb_bgi[(l, d)] = t_bg
                    t_bh = consts.tile([128, 2, BL], BF16, name=f"sb_bhn{l}{d}")
                    nc.sync.dma_start(out=t_bh, in_=bhn[(l, d)][:])
                    sb_bhn[(l, d)] = t_bh
            sb_wemb = consts.tile([128, 4, NOUT], BF16, name="sb_wemb")
            for k in range(4):
                nc.sync.dma_start(out=sb_wemb[:, k, :], in_=wemb[k])
            sb_bemb = consts.tile([NOUT, 1], F32, name="sb_bemb")
            nc.sync.dma_start(out=sb_bemb, in_=bemb[:])
            sb_id = consts.tile([128, 128], BF16, name="sb_id")
            nc.sync.dma_start(out=sb_id, in_=ident[:])
            zero2 = consts.tile([128, 2, BL], BF16, name="zero2")
            nc.vector.memset(zero2, 0.0)

            hb = None  # current layer's output state tiles

            copy_flip = [0]

            def emit_inproj_block(l, d, n, src):
                """gi block n for (layer l, dir d): 6 m-chunk GEMMs + copies.

                gi layout is t-major: [128, NT, NM, BL].
                """
                kin = 2 if l == 0 else 4
                blk = gipool.tile([128, NT, NM, BL], BF16,
                                  name=f"gi{l}{d}{n}", tag=f"gi_{d}{n}")
                for m in range(NM):
                    pt = ippool.tile([128, NT, BL], F32,
                                     name=f"ip{l}{d}{m}{n}", tag="ip")
                    for k in range(kin):
                        nc.tensor.matmul(
                            pt[:],
                            sb_wih[(l, d)][:, k, m * 128:(m + 1) * 128],
                            src(k, n),
                            start=(k == 0), stop=(k == kin - 1))
                    dst = blk[:, :, m, :]
                    if copy_flip[0] % 2 == 0:
                        nc.vector.tensor_scalar(
                            out=dst, in0=pt, scalar1=sb_bgi[(l, d)][:, m:m + 1],
                            scalar2=None, op0=OP.add)
                    else:
                        nc.scalar.activation(
                            out=dst, in_=pt, func=AF.Identity,
                            bias=sb_bgi[(l, d)][:, m:m + 1], scale=1.0)
                    copy_flip[0] += 1
                return blk

            for l in (0, 1):
                kin = 2 if l == 0 else 4
                if l == 0:
                    def src(k, n, _x=sb_x):
                        c0 = n * NCH
                        return _x[:, k, c0:c0 + NCH]
                else:
                    hb_prev = hb

                    def src(k, n, _h=hb_prev):
                        return _h[DIRS[k // 2]][n][:, :, k % 2, :]

                gi = {d: [None] * nb for d in DIRS}

                def emit_pair(j):
                    if j >= (nb + 1) // 2:
                        return
                    for d in DIRS:
                        for n in (j, nb - 1 - j) if j != nb - 1 - j else (j,):
                            if gi[d][n] is None:
                                gi[d][n] = emit_inproj_block(l, d, n, src)

                if l == 0:
                    emit_pair(0)
                    emit_pair(1)
                    ready_pairs = [2]
                else:
                    # layer-1 blocks gated by layer-0 hb availability;
                    # middle blocks finish first.
                    order, lo = [], (nb - 1) // 2
                    hi = lo + 1
                    while lo >= 0 or hi < nb:
                        if lo >= 0:
                            order.append(lo)
                        if hi < nb:
                            order.append(hi)
                        lo, hi = lo - 1, hi + 1
                    for n in order:
                        for d in DIRS:
                            gi[d][n] = emit_inproj_block(l, d, n, src)
                    ready_pairs = [nb]

                def gi_ap(d, t, m0, m1):
                    n = (t * BL) // NCH
                    tl = t - n * NT
                    return gi[d][n][:, tl, m0:m1, :]

                # ---- bidirectional scan (fwd and bwd anti-phase) ----
                # hb is t-major: [128, NT, 2, BL] per block, so per-step APs
                # are contiguous.
                hb = {}
                for d in DIRS:
                    hb[d] = [hpool.tile([128, NT, 2, BL], BF16,
                                        name=f"h{l}{d}{n}", tag=f"h_{d}{n}")
                             for n in range(nb)]

                def hb_ap(d, t):
                    n = (t * BL) // NCH
                    tl = t - n * NT
                    return hb[d][n][:, tl, :, :]

                def emit_inject(d, s):
                    # one PSUM bank per dir-step: gi_rz (64 cols) + bhn (32)
                    t = s if d == "f" else t_steps - 1 - s
                    ps = pspool.tile([128, NM, BL], F32, name=f"ps{l}{d}{s}",
                                     tag="scan", bufs=6)
                    nc.tensor.matmul(
                        ps[:, 0:4, :], sb_id[:], gi_ap(d, t, 0, 4),
                        start=True, stop=False)
                    nc.tensor.matmul(
                        ps[:, 4:6, :], sb_id[:], sb_bhn[(l, d)][:],
                        start=False, stop=False)
                    return ps

                ptiles = {d: emit_inject(d, 0) for d in DIRS}
                last_sig = {d: None for d in DIRS}
                for s in range(t_steps):
                    if s % NT == 0 and ready_pairs[0] <= s // NT + 2:
                        emit_pair(ready_pairs[0])
                        ready_pairs[0] += 1
                    for d in DIRS:
                        other = "b" if d == "f" else "f"
                        t = s if d == "f" else t_steps - 1 - s
                        ps = ptiles[d]
                        prz, pn = ps[:, 0:4, :], ps[:, 4:6, :]
                        # next step's injection first: runs contiguous with
                        # this step's w-matmuls on the PE (no restart bubble)
                        if s + 1 < t_steps:
                            nxt = emit_inject(d, s + 1)
                        if s == 0:
                            rhs = [zero2[:, 0, :], zero2[:, 1, :]]
                            hprev = zero2[:]
                        else:
                            tp = s - 1 if d == "f" else t_steps - s
                            hprev = hb_ap(d, tp)
                            rhs = [hprev[:, 0, :], hprev[:, 1, :]]
                        first_mm = None
                        for m in range(NM):
                            for k in range(2):
                                mm = nc.tensor.matmul(
                                    ps[:, m, :],
                                    sb_whh[(l, d)][:, k, m * 128:(m + 1) * 128],
                                    rhs[k], start=False, stop=(k == 1))
                                if first_mm is None:
                                    first_mm = mm
                        # anti-phase interlock: this dir's matmuls wait for
                        # the other dir's sigmoid of the previous half-step
                        if last_sig[other] is not None:
                            add_dep_helper(last_sig[other].ins, first_mm.ins,
                                           sync=True,
                                           reason="scan anti-phase interlock")
                        sg = work.tile([128, 4, BL], BF16, name=f"sg{l}{d}{s}",
                                       tag=f"sg_{d}")
                        sig = nc.scalar.activation(out=sg, in_=prz,
                                                   func=AF.Sigmoid)
                        last_sig[d] = sig
                        # n gate: nh = (gh_n + b_hh_n) * r + gi_n  (f32)
                        nh = work.tile([128, 2, BL], F32, name=f"nh{l}{d}{s}",
                                       tag=f"nh_{d}")
                        nc.vector.tensor_tensor(
                            out=nh, in0=pn, in1=sg[:, 0:2, :], op=OP.mult)
                        nh2 = work.tile([128, 2, BL], F32, name=f"nj{l}{d}{s}",
                                        tag=f"nj_{d}")
                        nc.vector.tensor_tensor(
                            out=nh2, in0=nh, in1=gi_ap(d, t, 4, 6), op=OP.add)
                        # off-critical-path: a = z'*h ; c = h - a  (= z*h)
                        av = work.tile([128, 2, BL], BF16, name=f"a{l}{d}{s}",
                                       tag=f"a_{d}")
                        nc.vector.tensor_tensor(
                            out=av, in0=sg[:, 2:4, :], in1=hprev, op=OP.mult)
                        cv = work.tile([128, 2, BL], BF16, name=f"c{l}{d}{s}",
                                       tag=f"c_{d}")
                        nc.vector.tensor_tensor(
                            out=cv, in0=hprev, in1=av, op=OP.subtract)
                        nt_ = work.tile([128, 2, BL], BF16, name=f"nt{l}{d}{s}",
                                        tag=f"nt_{d}")
                        nc.scalar.activation(out=nt_, in_=nh2, func=AF.Tanh)
                        # h' = z'*n + c, written bf16 straight into hb
                        m1 = work.tile([128, 2, BL], BF16, name=f"m{l}{d}{s}",
                                       tag=f"m_{d}")
                        nc.vector.tensor_tensor(
                            out=m1, in0=sg[:, 2:4, :], in1=nt_, op=OP.mult)
                        nc.vector.tensor_tensor(
                            out=hb_ap(d, t), in0=m1, in1=cv, op=OP.add)
                        if s + 1 < t_steps:
                            ptiles[d] = nxt

            # ---- final projection: outT = w_emb @ h2.T + b_emb ----
            eorder, lo = [], (nb - 1) // 2
            hi = lo + 1
            while lo >= 0 or hi < nb:
                if lo >= 0:
                    eorder.append(lo)
                if hi < nb:
                    eorder.append(hi)
                lo, hi = lo - 1, hi + 1
            for n in eorder:
                c0 = n * NCH
                pe = ippool.tile([NOUT, NCH], F32, name=f"pe{n}", tag="ip")
                for k in range(4):
                    nc.tensor.matmul(pe[:], sb_wemb[:, k, :],
                                     hb[DIRS[k // 2]][n][:, :, k % 2, :],
                                     start=(k == 0), stop=(k == 3))
                ob = work.tile([NOUT, NCH], F32, name=f"ob{n}", tag="ob", bufs=3)
                nc.scalar.activation(out=ob, in_=pe,
                                     func=AF.Identity, bias=sb_bemb[:, 0:1],
                                     scale=1.0)
                nc.sync.dma_start(out=outT[:, c0:c0 + NCH], in_=ob)

    nc.finalize()
    return nc


def _bf(a):
    return np.ascontiguousarray(a.astype(ml_dtypes.bfloat16))


def _f32(a):
    return np.ascontiguousarray(a.astype(np.float32))


def prep_shared(inputs, t_steps=T):
    """Host-side prep of the (core-independent) weight tensors.

    The z-gate rows (256:512) of w_ih, w_hh and the folded bias are negated
    so the device's sigmoid yields z' = 1 - z.
    """
    sh = {}
    for l in (0, 1):
        for d in DIRS:
            suf = f"l{l}{d}"
            w_ih = np.asarray(inputs[f"w_ih_{suf}"], np.float32).copy()
            w_hh = np.asarray(inputs[f"w_hh_{suf}"], np.float32).copy()
            b_ih = np.asarray(inputs[f"b_ih_{suf}"], np.float32)
            b_hh = np.asarray(inputs[f"b_hh_{suf}"], np.float32)
            w_ih[H:2 * H] *= -1.0
            w_hh[H:2 * H] *= -1.0
            kin = w_ih.shape[1] // 128
            sh[f"wih{l}{d}"] = _bf(w_ih.T.reshape(kin, 128, G3))
            sh[f"whh{l}{d}"] = _bf(w_hh.T.reshape(2, 128, G3))
            bg = b_ih.copy()
            bg[:2 * H] += b_hh[:2 * H]
            bg[H:2 * H] *= -1.0
            sh[f"bgi{l}{d}"] = _f32(bg.reshape(NM, 128).T)
            bhn_pc = b_hh[2 * H:].reshape(2, 128).T          # (128, 2)
            sh[f"bhn{l}{d}"] = _bf(
                np.broadcast_to(bhn_pc[:, :, None], (128, 2, BL)))
    w_emb = np.asarray(inputs["w_emb"], np.float32)                # (96, 512)
    sh["wemb"] = _bf(w_emb.T.reshape(4, 128, NOUT))
    sh["bemb"] = _f32(np.asarray(inputs["b_emb"], np.float32).reshape(NOUT, 1))
    sh["ident"] = _bf(np.eye(128, dtype=np.float32))
    return sh


def prep_in_maps(inputs, t_steps=T):
    x = np.asarray(inputs["x"], np.float32)[:t_steps]              # (T, B, NIN)
    sh = prep_shared(inputs, t_steps)
    tb = t_steps * BL
    in_maps = []
    for c in range(NCORES):
        xc = x[:, c * BL:(c + 1) * BL, :]                          # (T, BL, NIN)
        xT = xc.transpose(2, 0, 1).reshape(NIN, tb)                # (NIN, T*BL)
        m = dict(sh)
        m["xT"] = _bf(xT.reshape(2, 128, tb))
        in_maps.append(m)
    return in_maps


def assemble(results, t_steps=T):
    outs = []
    for c in range(NCORES):
        o = np.asarray(results[c]["outT"], np.float32)             # (96, T*BL)
        outs.append(o.reshape(NOUT, t_steps, BL).transpose(1, 2, 0))
    return np.concatenate(outs, axis=1)                            # (T, B, 96)


@functools.lru_cache(maxsize=2)
def get_nc(t_steps=T):
    return build_bass(t_steps)


_NEFF_CACHE = "/tmp/neff_cache_gru"


def _install_neff_cache():
    """Cache walrus-compiled NEFFs keyed by BIR content hash."""
    import hashlib
    import os
    import shutil
    import concourse.bass2jax as b2j
    if getattr(b2j, "_neff_cache_installed", False):
        return
    os.makedirs(_NEFF_CACHE, exist_ok=True)
    orig = b2j.compile_bir_kernel

    def cached(ant_bir_str, compile_dir_path, neff_name="file.neff", **kw):
        h = hashlib.sha256(ant_bir_str).hexdigest()[:24]
        cpath = os.path.join(_NEFF_CACHE, f"{h}.neff")
        dst = os.path.join(compile_dir_path, neff_name)
        if os.path.exists(cpath):
            shutil.copyfile(cpath, dst)
            return dst
        neff = orig(ant_bir_str, compile_dir_path, neff_name=neff_name, **kw)
        try:
            shutil.copyfile(neff, cpath)
        except OSError:
            pass
        return neff

    b2j.compile_bir_kernel = cached
    b2j._neff_cache_installed = True


def _install_ntff_hook():
    """Wire up the axon NTFF profile hook that this image's antenv lacks."""
    import types
    if "antenv.axon_hooks" not in sys.modules:
        mod = types.ModuleType("antenv.axon_hooks")
        holder = {}
        mod.set_axon_ntff_profile_hook = lambda h: holder.__setitem__("h", h)
        mod.get_axon_ntff_profile_hook = lambda: holder.get("h")
        sys.modules["antenv.axon_hooks"] = mod
        import antenv
        antenv.axon_hooks = mod
    else:
        mod = sys.modules["antenv.axon_hooks"]
    if mod.get_axon_ntff_profile_hook() is None:
        if "/root/.axon_site" not in sys.path:
            sys.path.insert(0, "/root/.axon_site")
        from trn_agent_boot.trn_boot import _ntff_profile_via_ctypes
        mod.set_axon_ntff_profile_hook(
            _ntff_profile_via_ctypes("/opt/axon/libaxon_pjrt.so"))
    import concourse.bass_utils as bu
    bu.upload_artifacts = lambda tmpdir: f"local:{tmpdir}"


def _run(inputs, t_steps=T, trace=False):
    from concourse.bass_utils import run_bass_kernel_spmd
    _install_neff_cache()
    if trace:
        _install_ntff_hook()
    nc = get_nc(t_steps)
    in_maps = prep_in_maps(inputs, t_steps)
    res = run_bass_kernel_spmd(nc, in_maps, list(range(NCORES)), trace=trace)
    return assemble(res.results, t_steps), res


def kernel(**inputs):
    out, _ = _run(inputs, T, trace=False)
    return out


def run_traced(inputs, t_steps=T):
    out, res = _run(inputs, t_steps, trace=True)
    trace_path = None
    if res.instructions_and_trace is not None:
        trace_path = res.instructions_and_trace[1]
    return out, res.exec_time_ns, trace_path


# revision 7
# speedup vs baseline: 1.4844x; 1.4844x over previous
"""Trainium2 Bass kernel for a 2-layer bidirectional GRU + linear head.

Problem: nn_BidirectionalGRU (T=256, B=128, NIN=256, H=256, NOUT=96).

Strategy (8 NeuronCores, data-parallel over batch, 16 rows/core):
  - Gate-major layout: feature dims on SBUF partitions, (time*batch) on the
    free dim. gi and h are stored t-major so every per-step AP is contiguous
    (DVE 2x mode) and scan injection is one 64-column identity matmul.
  - The z-gate rows of all weights/biases are negated on the host so sigmoid
    yields z' = 1-z; then h' = z'*n + (h - z'*h) and the two products
    a = z'*h, c = h - a are computed while tanh runs, leaving only two DVE
    ops after tanh on the critical path.
  - fwd/bwd scans are locked in anti-phase via explicit cross-dependencies
    (each dir's w-matmuls wait on the other dir's sigmoid) so the gate chain
    of one dir overlaps the PE phase of the other.
  - Inproj blocks (N=256) are emitted just-in-time between scan steps.
"""

import functools
import sys

import numpy as np

sys.path.insert(0, "/opt/trn_rl_repo")

import ml_dtypes  # noqa: E402
import concourse.bass as bass  # noqa: E402
import concourse.tile as tile  # noqa: E402
from concourse import bacc, mybir  # noqa: E402
from concourse.tile_rust import add_dep_helper  # noqa: E402

T, B, NIN, H, NOUT = 256, 128, 256, 256, 96
NCORES = 8
BL = B // NCORES          # 16 batch rows per core
G3 = 3 * H                # 768 gate rows
NM = G3 // 128            # 6 gate-row chunks
AF = mybir.ActivationFunctionType
OP = mybir.AluOpType
BF16, F32 = mybir.dt.bfloat16, mybir.dt.float32
NCH = 256                 # inproj streaming chunk (columns)
NT = NCH // BL            # t-steps per inproj block (16)

DIRS = ("f", "b")


def build_bass(t_steps=T):
    """Build the per-core Bass program (identical on all cores)."""
    tb = t_steps * BL
    nb = tb // NCH
    nc = bacc.Bacc(None, target_bir_lowering=False, debug=False)

    xT = nc.declare_dram_parameter("xT", [2, 128, tb], BF16, isOutput=False)
    ident = nc.declare_dram_parameter("ident", [128, 128], BF16, isOutput=False)
    wih, whh, bgi, bhn = {}, {}, {}, {}
    for l in (0, 1):
        kin = 2 if l == 0 else 4
        for d in DIRS:
            wih[(l, d)] = nc.declare_dram_parameter(
                f"wih{l}{d}", [kin, 128, G3], BF16, isOutput=False)
            whh[(l, d)] = nc.declare_dram_parameter(
                f"whh{l}{d}", [2, 128, G3], BF16, isOutput=False)
            bgi[(l, d)] = nc.declare_dram_parameter(
                f"bgi{l}{d}", [128, NM], F32, isOutput=False)
            bhn[(l, d)] = nc.declare_dram_parameter(
                f"bhn{l}{d}", [128, 2, BL], BF16, isOutput=False)
    wemb = nc.declare_dram_parameter("wemb", [4, 128, NOUT], BF16, isOutput=False)
    bemb = nc.declare_dram_parameter("bemb", [NOUT, 1], F32, isOutput=False)
    outT = nc.declare_dram_parameter("outT", [NOUT, tb], F32, isOutput=True)

    with tile.TileContext(nc) as tc:
        from contextlib import ExitStack
        with ExitStack() as ctx:
            consts = ctx.enter_context(tc.tile_pool(name="consts", bufs=1))
            hpool = ctx.enter_context(tc.tile_pool(name="hstate", bufs=1))
            gipool = ctx.enter_context(tc.tile_pool(name="gi", bufs=1))
            pspool = ctx.enter_context(tc.tile_pool(name="scanps", bufs=3, space="PSUM"))
            ippool = ctx.enter_context(tc.tile_pool(name="ips", bufs=2, space="PSUM"))
            work = ctx.enter_context(tc.tile_pool(name="work", bufs=4))

            # ---- load constants ----
            sb_x = consts.tile([128, 2, tb], BF16, name="sb_x")
            for k in range(2):
                nc.sync.dma_start(out=sb_x[:, k, :], in_=xT[k])
            sb_wih, sb_whh, sb_bgi, sb_bhn = {}, {}, {}, {}
            for l in (0, 1):
                kin = 2 if l == 0 else 4
                for d in DIRS:
                    t_ih = consts.tile([128, kin, G3], BF16, name=f"sb_wih{l}{d}")
                    for k in range(kin):
                        nc.sync.dma_start(out=t_ih[:, k, :], in_=wih[(l, d)][k])
                    sb_wih[(l, d)] = t_ih
                    t_hh = consts.tile([128, 2, G3], BF16, name=f"sb_whh{l}{d}")
                    for k in range(2):
                        nc.sync.dma_start(out=t_hh[:, k, :], in_=whh[(l, d)][k])
                    sb_whh[(l, d)] = t_hh
                    t_bg = consts.tile([128, NM], F32, name=f"sb_bgi{l}{d}")
                    nc.sync.dma_start(out=t_bg, in_=bgi[(l, d)][:])
                    sb_bgi[(l, d)] = t_bg
                    t_bh = consts.tile([128, 2, BL], BF16, name=f"sb_bhn{l}{d}")
                    nc.sync.dma_start(out=t_bh, in_=bhn[(l, d)][:])
                    sb_bhn[(l, d)] = t_bh
            sb_wemb = consts.tile([128, 4, NOUT], BF16, name="sb_wemb")
            for k in range(4):
                nc.sync.dma_start(out=sb_wemb[:, k, :], in_=wemb[k])
            sb_bemb = consts.tile([NOUT, 1], F32, name="sb_bemb")
            nc.sync.dma_start(out=sb_bemb, in_=bemb[:])
            sb_id = consts.tile([128, 128], BF16, name="sb_id")
            nc.sync.dma_start(out=sb_id, in_=ident[:])
            zero2 = consts.tile([128, 2, BL], BF16, name="zero2")
            nc.vector.memset(zero2, 0.0)

            hb = None  # current layer's output state tiles

            copy_flip = [0]

            def emit_inproj_block(l, d, n, src):
                """gi block n for (layer l, dir d): 6 m-chunk GEMMs + copies.

                gi layout is t-major: [128, NT, NM, BL].
                """
                kin = 2 if l == 0 else 4
                blk = gipool.tile([128, NT, NM, BL], BF16,
                                  name=f"gi{l}{d}{n}", tag=f"gi_{d}{n}")
                for m in range(NM):
                    pt = ippool.tile([128, NT, BL], F32,
                                     name=f"ip{l}{d}{m}{n}", tag="ip")
                    for k in range(kin):
                        nc.tensor.matmul(
                            pt[:],
                            sb_wih[(l, d)][:, k, m * 128:(m + 1) * 128],
                            src(k, n),
                            start=(k == 0), stop=(k == kin - 1))
                    dst = blk[:, :, m, :]
                    if copy_flip[0] % 2 == 0:
                        nc.vector.tensor_scalar(
                            out=dst, in0=pt, scalar1=sb_bgi[(l, d)][:, m:m + 1],
                            scalar2=None, op0=OP.add)
                    else:
                        nc.scalar.activation(
                            out=dst, in_=pt, func=AF.Identity,
                            bias=sb_bgi[(l, d)][:, m:m + 1], scale=1.0)
                    copy_flip[0] += 1
                return blk

            for l in (0, 1):
                kin = 2 if l == 0 else 4
                if l == 0:
                    def src(k, n, _x=sb_x):
                        c0 = n * NCH
                        return _x[:, k, c0:c0 + NCH]
                else:
                    hb_prev = hb

                    def src(k, n, _h=hb_prev):
                        return _h[DIRS[k // 2]][n][:, :, k % 2, :]

                gi = {d: [None] * nb for d in DIRS}

                def emit_pair(j):
                    if j >= (nb + 1) // 2:
                        return
                    for d in DIRS:
                        for n in (j, nb - 1 - j) if j != nb - 1 - j else (j,):
                            if gi[d][n] is None:
                                gi[d][n] = emit_inproj_block(l, d, n, src)

                if l == 0:
                    emit_pair(0)
                    emit_pair(1)
                    ready_pairs = [2]
                else:
                    # layer-1 blocks gated by layer-0 hb availability;
                    # middle blocks finish first.
                    order, lo = [], (nb - 1) // 2
                    hi = lo + 1
                    while lo >= 0 or hi < nb:
                        if lo >= 0:
                            order.append(lo)
                        if hi < nb:
                            order.append(hi)
                        lo, hi = lo - 1, hi + 1
                    for n in order:
                        for d in DIRS:
                            gi[d][n] = emit_inproj_block(l, d, n, src)
                    ready_pairs = [nb]

                def gi_ap(d, t, m0, m1):
                    n = (t * BL) // NCH
                    tl = t - n * NT
                    return gi[d][n][:, tl, m0:m1, :]

                # ---- bidirectional scan (fwd and bwd anti-phase) ----
                # hb is t-major: [128, NT, 2, BL] per block, so per-step APs
                # are contiguous.
                hb = {}
                for d in DIRS:
                    hb[d] = [hpool.tile([128, NT, 2, BL], BF16,
                                        name=f"h{l}{d}{n}", tag=f"h_{d}{n}")
                             for n in range(nb)]

                def hb_ap(d, t):
                    n = (t * BL) // NCH
                    tl = t - n * NT
                    return hb[d][n][:, tl, :, :]

                def emit_inject(d, s):
                    # one PSUM bank per dir-step: gi_rz (64 cols) + bhn (32)
                    t = s if d == "f" else t_steps - 1 - s
                    ps = pspool.tile([128, NM, BL], F32, name=f"ps{l}{d}{s}",
                                     tag="scan", bufs=6)
                    nc.tensor.matmul(
                        ps[:, 0:4, :], sb_id[:], gi_ap(d, t, 0, 4),
                        start=True, stop=False)
                    nc.tensor.matmul(
                        ps[:, 4:6, :], sb_id[:], sb_bhn[(l, d)][:],
                        start=False, stop=False)
                    return ps

                ptiles = {d: emit_inject(d, 0) for d in DIRS}
                last_sig = {d: None for d in DIRS}
                for s in range(t_steps):
                    if s % NT == 0 and ready_pairs[0] <= s // NT + 2:
                        emit_pair(ready_pairs[0])
                        ready_pairs[0] += 1
                    for d in DIRS:
                        other = "b" if d == "f" else "f"
                        t = s if d == "f" else t_steps - 1 - s
                        ps = ptiles[d]
                        prz, pn = ps[:, 0:4, :], ps[:, 4:6, :]
                        # next step's injection first: runs contiguous with
                        # this step's w-matmuls on the PE (no restart bubble)
                        if s + 1 < t_steps:
                            nxt = emit_inject(d, s + 1)
                        if s == 0:
                            rhs = [zero2[:, 0, :], zero2[:, 1, :]]
                            hprev = zero2[:]
                        else:
                            tp = s - 1 if d == "f" else t_steps - s
                            hprev = hb_ap(d, tp)
                            rhs = [hprev[:, 0, :], hprev[:, 1, :]]
                        first_mm = None
                        for m in range(NM):
                            for k in range(2):
                                mm = nc.tensor.matmul(
                                    ps[:, m, :],
                                    sb_whh[(l, d)][:, k, m * 128:(m + 1) * 128],
                                    rhs[k], start=False, stop=(k == 1))
                                if first_mm is None:
                                    first_mm = mm
                        # anti-phase interlock: this dir's matmuls wait for
                        # the other dir's sigmoid of the previous half-step
                        if last_sig[other] is not None:
                            add_dep_helper(last_sig[other].ins, first_mm.ins,
                                           sync=False,
                                           reason="scan anti-phase interlock")
                        sg = work.tile([128, 4, BL], BF16, name=f"sg{l}{d}{s}",
                                       tag=f"sg_{d}")
                        sig = nc.scalar.activation(out=sg, in_=prz,
                                                   func=AF.Sigmoid)
                        last_sig[d] = sig
                        # n gate: nh = (gh_n + b_hh_n) * r + gi_n  (f32)
                        nh = work.tile([128, 2, BL], F32, name=f"nh{l}{d}{s}",
                                       tag=f"nh_{d}")
                        nc.vector.tensor_tensor(
                            out=nh, in0=pn, in1=sg[:, 0:2, :], op=OP.mult)
                        nh2 = work.tile([128, 2, BL], F32, name=f"nj{l}{d}{s}",
                                        tag=f"nj_{d}")
                        nc.vector.tensor_tensor(
                            out=nh2, in0=nh, in1=gi_ap(d, t, 4, 6), op=OP.add)
                        # off-critical-path: a = z'*h ; c = h - a  (= z*h)
                        av = work.tile([128, 2, BL], BF16, name=f"a{l}{d}{s}",
                                       tag=f"a_{d}")
                        nc.vector.tensor_tensor(
                            out=av, in0=sg[:, 2:4, :], in1=hprev, op=OP.mult)
                        cv = work.tile([128, 2, BL], BF16, name=f"c{l}{d}{s}",
                                       tag=f"c_{d}")
                        nc.vector.tensor_tensor(
                            out=cv, in0=hprev, in1=av, op=OP.subtract)
                        nt_ = work.tile([128, 2, BL], BF16, name=f"nt{l}{d}{s}",
                                        tag=f"nt_{d}")
                        nc.scalar.activation(out=nt_, in_=nh2, func=AF.Tanh)
                        # h' = z'*n + c, written bf16 straight into hb
                        m1 = work.tile([128, 2, BL], BF16, name=f"m{l}{d}{s}",
                                       tag=f"m_{d}")
                        nc.vector.tensor_tensor(
                            out=m1, in0=sg[:, 2:4, :], in1=nt_, op=OP.mult)
                        nc.vector.tensor_tensor(
                            out=hb_ap(d, t), in0=m1, in1=cv, op=OP.add)
                        if s + 1 < t_steps:
                            ptiles[d] = nxt

            # ---- final projection: outT = w_emb @ h2.T + b_emb ----
            eorder, lo = [], (nb - 1) // 2
            hi = lo + 1
            while lo >= 0 or hi < nb:
                if lo >= 0:
                    eorder.append(lo)
                if hi < nb:
                    eorder.append(hi)
                lo, hi = lo - 1, hi + 1
            for n in eorder:
                c0 = n * NCH
                pe = ippool.tile([NOUT, NCH], F32, name=f"pe{n}", tag="ip")
                for k in range(4):
                    nc.tensor.matmul(pe[:], sb_wemb[:, k, :],
                                     hb[DIRS[k // 2]][n][:, :, k % 2, :],
                                     start=(k == 0), stop=(k == 3))
                ob = work.tile([NOUT, NCH], F32, name=f"ob{n}", tag="ob", bufs=3)
                nc.scalar.activation(out=ob, in_=pe,
                                     func=AF.Identity, bias=sb_bemb[:, 0:1],
                                     scale=1.0)
                nc.sync.dma_start(out=outT[:, c0:c0 + NCH], in_=ob)

    nc.finalize()
    return nc


def _bf(a):
    return np.ascontiguousarray(a.astype(ml_dtypes.bfloat16))


def _f32(a):
    return np.ascontiguousarray(a.astype(np.float32))


def prep_shared(inputs, t_steps=T):
    """Host-side prep of the (core-independent) weight tensors.

    The z-gate rows (256:512) of w_ih, w_hh and the folded bias are negated
    so the device's sigmoid yields z' = 1 - z.
    """
    sh = {}
    for l in (0, 1):
        for d in DIRS:
            suf = f"l{l}{d}"
            w_ih = np.asarray(inputs[f"w_ih_{suf}"], np.float32).copy()
            w_hh = np.asarray(inputs[f"w_hh_{suf}"], np.float32).copy()
            b_ih = np.asarray(inputs[f"b_ih_{suf}"], np.float32)
            b_hh = np.asarray(inputs[f"b_hh_{suf}"], np.float32)
            w_ih[H:2 * H] *= -1.0
            w_hh[H:2 * H] *= -1.0
            kin = w_ih.shape[1] // 128
            sh[f"wih{l}{d}"] = _bf(w_ih.T.reshape(kin, 128, G3))
            sh[f"whh{l}{d}"] = _bf(w_hh.T.reshape(2, 128, G3))
            bg = b_ih.copy()
            bg[:2 * H] += b_hh[:2 * H]
            bg[H:2 * H] *= -1.0
            sh[f"bgi{l}{d}"] = _f32(bg.reshape(NM, 128).T)
            bhn_pc = b_hh[2 * H:].reshape(2, 128).T          # (128, 2)
            sh[f"bhn{l}{d}"] = _bf(
                np.broadcast_to(bhn_pc[:, :, None], (128, 2, BL)))
    w_emb = np.asarray(inputs["w_emb"], np.float32)                # (96, 512)
    sh["wemb"] = _bf(w_emb.T.reshape(4, 128, NOUT))
    sh["bemb"] = _f32(np.asarray(inputs["b_emb"], np.float32).reshape(NOUT, 1))
    sh["ident"] = _bf(np.eye(128, dtype=np.float32))
    return sh


def prep_in_maps(inputs, t_steps=T):
    x = np.asarray(inputs["x"], np.float32)[:t_steps]              # (T, B, NIN)
    sh = prep_shared(inputs, t_steps)
    tb = t_steps * BL
    in_maps = []
    for c in range(NCORES):
        xc = x[:, c * BL:(c + 1) * BL, :]                          # (T, BL, NIN)
        xT = xc.transpose(2, 0, 1).reshape(NIN, tb)                # (NIN, T*BL)
        m = dict(sh)
        m["xT"] = _bf(xT.reshape(2, 128, tb))
        in_maps.append(m)
    return in_maps


def assemble(results, t_steps=T):
    outs = []
    for c in range(NCORES):
        o = np.asarray(results[c]["outT"], np.float32)             # (96, T*BL)
        outs.append(o.reshape(NOUT, t_steps, BL).transpose(1, 2, 0))
    return np.concatenate(outs, axis=1)                            # (T, B, 96)


@functools.lru_cache(maxsize=2)
def get_nc(t_steps=T):
    return build_bass(t_steps)


_NEFF_CACHE = "/tmp/neff_cache_gru"


def _install_neff_cache():
    """Cache walrus-compiled NEFFs keyed by BIR content hash."""
    import hashlib
    import os
    import shutil
    import concourse.bass2jax as b2j
    if getattr(b2j, "_neff_cache_installed", False):
        return
    os.makedirs(_NEFF_CACHE, exist_ok=True)
    orig = b2j.compile_bir_kernel

    def cached(ant_bir_str, compile_dir_path, neff_name="file.neff", **kw):
        h = hashlib.sha256(ant_bir_str).hexdigest()[:24]
        cpath = os.path.join(_NEFF_CACHE, f"{h}.neff")
        dst = os.path.join(compile_dir_path, neff_name)
        if os.path.exists(cpath):
            shutil.copyfile(cpath, dst)
            return dst
        neff = orig(ant_bir_str, compile_dir_path, neff_name=neff_name, **kw)
        try:
            shutil.copyfile(neff, cpath)
        except OSError:
            pass
        return neff

    b2j.compile_bir_kernel = cached
    b2j._neff_cache_installed = True


def _install_ntff_hook():
    """Wire up the axon NTFF profile hook that this image's antenv lacks."""
    import types
    if "antenv.axon_hooks" not in sys.modules:
        mod = types.ModuleType("antenv.axon_hooks")
        holder = {}
        mod.set_axon_ntff_profile_hook = lambda h: holder.__setitem__("h", h)
        mod.get_axon_ntff_profile_hook = lambda: holder.get("h")
        sys.modules["antenv.axon_hooks"] = mod
        import antenv
        antenv.axon_hooks = mod
    else:
        mod = sys.modules["antenv.axon_hooks"]
    if mod.get_axon_ntff_profile_hook() is None:
        if "/root/.axon_site" not in sys.path:
            sys.path.insert(0, "/root/.axon_site")
        from trn_agent_boot.trn_boot import _ntff_profile_via_ctypes
        mod.set_axon_ntff_profile_hook(
            _ntff_profile_via_ctypes("/opt/axon/libaxon_pjrt.so"))
    import concourse.bass_utils as bu
    bu.upload_artifacts = lambda tmpdir: f"local:{tmpdir}"


def _run(inputs, t_steps=T, trace=False):
    from concourse.bass_utils import run_bass_kernel_spmd
    _install_neff_cache()
    if trace:
        _install_ntff_hook()
    nc = get_nc(t_steps)
    in_maps = prep_in_maps(inputs, t_steps)
    res = run_bass_kernel_spmd(nc, in_maps, list(range(NCORES)), trace=trace)
    return assemble(res.results, t_steps), res


def kernel(**inputs):
    out, _ = _run(inputs, T, trace=False)
    return out


def run_traced(inputs, t_steps=T):
    out, res = _run(inputs, t_steps, trace=True)
    trace_path = None
    if res.instructions_and_trace is not None:
        trace_path = res.instructions_and_trace[1]
    return out, res.exec_time_ns, trace_path


# revision 8
# speedup vs baseline: 1.4889x; 1.0030x over previous
"""Trainium2 Bass kernel for a 2-layer bidirectional GRU + linear head.

Problem: nn_BidirectionalGRU (T=256, B=128, NIN=256, H=256, NOUT=96).

Strategy (8 NeuronCores, data-parallel over batch, 16 rows/core):
  - Gate-major layout: feature dims on SBUF partitions, (time*batch) on the
    free dim. gi and h are stored t-major so every per-step AP is contiguous
    (DVE 2x mode) and scan injection is one 64-column identity matmul.
  - The z-gate rows of all weights/biases are negated on the host so sigmoid
    yields z' = 1-z; then h' = z'*n + (h - z'*h) and the two products
    a = z'*h, c = h - a are computed while tanh runs, leaving only two DVE
    ops after tanh on the critical path.
  - fwd/bwd scans are locked in anti-phase via explicit cross-dependencies
    (each dir's w-matmuls wait on the other dir's sigmoid) so the gate chain
    of one dir overlaps the PE phase of the other.
  - Inproj blocks (N=256) are emitted just-in-time between scan steps.
"""

import functools
import sys

import numpy as np

sys.path.insert(0, "/opt/trn_rl_repo")

import ml_dtypes  # noqa: E402
import concourse.bass as bass  # noqa: E402
import concourse.tile as tile  # noqa: E402
from concourse import bacc, mybir  # noqa: E402
from concourse.tile_rust import add_dep_helper  # noqa: E402

T, B, NIN, H, NOUT = 256, 128, 256, 256, 96
NCORES = 8
BL = B // NCORES          # 16 batch rows per core
G3 = 3 * H                # 768 gate rows
NM = G3 // 128            # 6 gate-row chunks
AF = mybir.ActivationFunctionType
OP = mybir.AluOpType
BF16, F32 = mybir.dt.bfloat16, mybir.dt.float32
NCH = 256                 # inproj streaming chunk (columns)
NT = NCH // BL            # t-steps per inproj block (16)

DIRS = ("f", "b")


def build_bass(t_steps=T):
    """Build the per-core Bass program (identical on all cores)."""
    tb = t_steps * BL
    nb = tb // NCH
    nc = bacc.Bacc(None, target_bir_lowering=False, debug=False)

    xT = nc.declare_dram_parameter("xT", [2, 128, tb], BF16, isOutput=False)
    ident = nc.declare_dram_parameter("ident", [128, 128], BF16, isOutput=False)
    wih, whh, bgi, bhn = {}, {}, {}, {}
    for l in (0, 1):
        kin = 2 if l == 0 else 4
        for d in DIRS:
            wih[(l, d)] = nc.declare_dram_parameter(
                f"wih{l}{d}", [kin, 128, G3], BF16, isOutput=False)
            whh[(l, d)] = nc.declare_dram_parameter(
                f"whh{l}{d}", [2, 128, G3], BF16, isOutput=False)
            bgi[(l, d)] = nc.declare_dram_parameter(
                f"bgi{l}{d}", [128, NM], F32, isOutput=False)
            bhn[(l, d)] = nc.declare_dram_parameter(
                f"bhn{l}{d}", [128, 2, BL], BF16, isOutput=False)
    wemb = nc.declare_dram_parameter("wemb", [4, 128, NOUT], BF16, isOutput=False)
    bemb = nc.declare_dram_parameter("bemb", [NOUT, 1], F32, isOutput=False)
    outT = nc.declare_dram_parameter("outT", [NOUT, tb], F32, isOutput=True)

    with tile.TileContext(nc) as tc:
        from contextlib import ExitStack
        with ExitStack() as ctx:
            consts = ctx.enter_context(tc.tile_pool(name="consts", bufs=1))
            hpool = ctx.enter_context(tc.tile_pool(name="hstate", bufs=1))
            gipool = ctx.enter_context(tc.tile_pool(name="gi", bufs=1))
            pspool = ctx.enter_context(tc.tile_pool(name="scanps", bufs=3, space="PSUM"))
            ippool = ctx.enter_context(tc.tile_pool(name="ips", bufs=2, space="PSUM"))
            work = ctx.enter_context(tc.tile_pool(name="work", bufs=4))

            # ---- load constants ----
            sb_x = consts.tile([128, 2, tb], BF16, name="sb_x")
            for k in range(2):
                nc.sync.dma_start(out=sb_x[:, k, :], in_=xT[k])
            sb_wih, sb_whh, sb_bgi, sb_bhn = {}, {}, {}, {}
            for l in (0, 1):
                kin = 2 if l == 0 else 4
                for d in DIRS:
                    t_ih = consts.tile([128, kin, G3], BF16, name=f"sb_wih{l}{d}")
                    for k in range(kin):
                        nc.sync.dma_start(out=t_ih[:, k, :], in_=wih[(l, d)][k])
                    sb_wih[(l, d)] = t_ih
                    t_hh = consts.tile([128, 2, G3], BF16, name=f"sb_whh{l}{d}")
                    for k in range(2):
                        nc.sync.dma_start(out=t_hh[:, k, :], in_=whh[(l, d)][k])
                    sb_whh[(l, d)] = t_hh
                    t_bg = consts.tile([128, NM], F32, name=f"sb_bgi{l}{d}")
                    nc.sync.dma_start(out=t_bg, in_=bgi[(l, d)][:])
                    sb_bgi[(l, d)] = t_bg
                    t_bh = consts.tile([128, 2, BL], BF16, name=f"sb_bhn{l}{d}")
                    nc.sync.dma_start(out=t_bh, in_=bhn[(l, d)][:])
                    sb_bhn[(l, d)] = t_bh
            sb_wemb = consts.tile([128, 4, NOUT], BF16, name="sb_wemb")
            for k in range(4):
                nc.sync.dma_start(out=sb_wemb[:, k, :], in_=wemb[k])
            sb_bemb = consts.tile([NOUT, 1], F32, name="sb_bemb")
            nc.sync.dma_start(out=sb_bemb, in_=bemb[:])
            sb_id = consts.tile([128, 128], BF16, name="sb_id")
            nc.sync.dma_start(out=sb_id, in_=ident[:])
            zero2 = consts.tile([128, 2, BL], BF16, name="zero2")
            nc.vector.memset(zero2, 0.0)

            hb = None  # current layer's output state tiles

            copy_flip = [0]

            def emit_inproj_block(l, d, n, src):
                """gi block n for (layer l, dir d): 6 m-chunk GEMMs + copies.

                gi layout is t-major: [128, NT, NM, BL].
                """
                kin = 2 if l == 0 else 4
                blk = gipool.tile([128, NT, NM, BL], BF16,
                                  name=f"gi{l}{d}{n}", tag=f"gi_{d}{n}")
                for m in range(NM):
                    pt = ippool.tile([128, NT, BL], F32,
                                     name=f"ip{l}{d}{m}{n}", tag="ip")
                    for k in range(kin):
                        nc.tensor.matmul(
                            pt[:],
                            sb_wih[(l, d)][:, k, m * 128:(m + 1) * 128],
                            src(k, n),
                            start=(k == 0), stop=(k == kin - 1))
                    dst = blk[:, :, m, :]
                    if copy_flip[0] % 2 == 0:
                        nc.vector.tensor_scalar(
                            out=dst, in0=pt, scalar1=sb_bgi[(l, d)][:, m:m + 1],
                            scalar2=None, op0=OP.add)
                    else:
                        nc.scalar.activation(
                            out=dst, in_=pt, func=AF.Identity,
                            bias=sb_bgi[(l, d)][:, m:m + 1], scale=1.0)
                    copy_flip[0] += 1
                return blk

            for l in (0, 1):
                kin = 2 if l == 0 else 4
                if l == 0:
                    def src(k, n, _x=sb_x):
                        c0 = n * NCH
                        return _x[:, k, c0:c0 + NCH]
                else:
                    hb_prev = hb

                    def src(k, n, _h=hb_prev):
                        return _h[DIRS[k // 2]][n][:, :, k % 2, :]

                gi = {d: [None] * nb for d in DIRS}

                def emit_pair(j):
                    if j >= (nb + 1) // 2:
                        return
                    for d in DIRS:
                        for n in (j, nb - 1 - j) if j != nb - 1 - j else (j,):
                            if gi[d][n] is None:
                                gi[d][n] = emit_inproj_block(l, d, n, src)

                if l == 0:
                    emit_pair(0)
                    emit_pair(1)
                    ready_pairs = [2]
                else:
                    # layer-1 blocks gated by layer-0 hb availability;
                    # middle blocks finish first.
                    order, lo = [], (nb - 1) // 2
                    hi = lo + 1
                    while lo >= 0 or hi < nb:
                        if lo >= 0:
                            order.append(lo)
                        if hi < nb:
                            order.append(hi)
                        lo, hi = lo - 1, hi + 1
                    for n in order:
                        for d in DIRS:
                            gi[d][n] = emit_inproj_block(l, d, n, src)
                    ready_pairs = [nb]

                def gi_ap(d, t, m0, m1):
                    n = (t * BL) // NCH
                    tl = t - n * NT
                    return gi[d][n][:, tl, m0:m1, :]

                # ---- bidirectional scan (fwd and bwd anti-phase) ----
                # hb is t-major: [128, NT, 2, BL] per block, so per-step APs
                # are contiguous.
                hb = {}
                for d in DIRS:
                    hb[d] = [hpool.tile([128, NT, 2, BL], BF16,
                                        name=f"h{l}{d}{n}", tag=f"h_{d}{n}")
                             for n in range(nb)]

                def hb_ap(d, t):
                    n = (t * BL) // NCH
                    tl = t - n * NT
                    return hb[d][n][:, tl, :, :]

                def emit_inject(d, s):
                    # one PSUM bank per dir-step: gi_rz (64 cols) + bhn (32)
                    t = s if d == "f" else t_steps - 1 - s
                    ps = pspool.tile([128, NM, BL], F32, name=f"ps{l}{d}{s}",
                                     tag="scan", bufs=6)
                    nc.tensor.matmul(
                        ps[:, 0:4, :], sb_id[:], gi_ap(d, t, 0, 4),
                        start=True, stop=False)
                    nc.tensor.matmul(
                        ps[:, 4:6, :], sb_id[:], sb_bhn[(l, d)][:],
                        start=False, stop=False)
                    return ps

                ptiles = {d: emit_inject(d, 0) for d in DIRS}
                last_sig = {d: None for d in DIRS}
                for s in range(t_steps):
                    if s % NT == 0 and ready_pairs[0] <= s // NT + 2:
                        emit_pair(ready_pairs[0])
                        ready_pairs[0] += 1
                    # quarter-offset emission: both dirs' MM+sigmoid phases
                    # first, then both gate chains, so the ACT queue order is
                    # [sig_f, sig_b, tanh_f, tanh_b] per step.
                    st = {}
                    for d in DIRS:
                        other = "b" if d == "f" else "f"
                        t = s if d == "f" else t_steps - 1 - s
                        ps = ptiles[d]
                        if s + 1 < t_steps:
                            nxt = emit_inject(d, s + 1)
                        else:
                            nxt = None
                        if s == 0:
                            hprev = zero2[:]
                        else:
                            tp = s - 1 if d == "f" else t_steps - s
                            hprev = hb_ap(d, tp)
                        rhs = [hprev[:, 0, :], hprev[:, 1, :]]
                        first_mm = None
                        for m in range(NM):
                            for k in range(2):
                                mm = nc.tensor.matmul(
                                    ps[:, m, :],
                                    sb_whh[(l, d)][:, k, m * 128:(m + 1) * 128],
                                    rhs[k], start=False, stop=(k == 1))
                                if first_mm is None:
                                    first_mm = mm
                        # anti-phase interlock (plan-order only)
                        if last_sig[other] is not None:
                            add_dep_helper(last_sig[other].ins, first_mm.ins,
                                           sync=False,
                                           reason="scan anti-phase interlock")
                        sg = work.tile([128, 4, BL], BF16, name=f"sg{l}{d}{s}",
                                       tag=f"sg_{d}")
                        sig = nc.scalar.activation(out=sg, in_=ps[:, 0:4, :],
                                                   func=AF.Sigmoid)
                        last_sig[d] = sig
                        st[d] = (ps, sg, hprev, t, nxt)
                    for d in DIRS:
                        ps, sg, hprev, t, nxt = st[d]
                        pn = ps[:, 4:6, :]
                        # n gate: nh = (gh_n + b_hh_n) * r + gi_n  (f32)
                        nh = work.tile([128, 2, BL], F32, name=f"nh{l}{d}{s}",
                                       tag=f"nh_{d}")
                        nc.vector.tensor_tensor(
                            out=nh, in0=pn, in1=sg[:, 0:2, :], op=OP.mult)
                        nh2 = work.tile([128, 2, BL], F32, name=f"nj{l}{d}{s}",
                                        tag=f"nj_{d}")
                        nc.vector.tensor_tensor(
                            out=nh2, in0=nh, in1=gi_ap(d, t, 4, 6), op=OP.add)
                        nt_ = work.tile([128, 2, BL], BF16, name=f"nt{l}{d}{s}",
                                        tag=f"nt_{d}")
                        nc.scalar.activation(out=nt_, in_=nh2, func=AF.Tanh)
                        st[d] = (ps, sg, hprev, t, nxt, nt_)
                    for d in DIRS:
                        ps, sg, hprev, t, nxt, nt_ = st[d]
                        # a = z'*h ; c = h - a  (= z*h): fill DVE during tanh
                        av = work.tile([128, 2, BL], BF16, name=f"a{l}{d}{s}",
                                       tag=f"a_{d}")
                        nc.vector.tensor_tensor(
                            out=av, in0=sg[:, 2:4, :], in1=hprev, op=OP.mult)
                        cv = work.tile([128, 2, BL], BF16, name=f"c{l}{d}{s}",
                                       tag=f"c_{d}")
                        nc.vector.tensor_tensor(
                            out=cv, in0=hprev, in1=av, op=OP.subtract)
                        # h' = z'*n + c, written bf16 straight into hb
                        m1 = work.tile([128, 2, BL], BF16, name=f"m{l}{d}{s}",
                                       tag=f"m_{d}")
                        nc.vector.tensor_tensor(
                            out=m1, in0=sg[:, 2:4, :], in1=nt_, op=OP.mult)
                        nc.vector.tensor_tensor(
                            out=hb_ap(d, t), in0=m1, in1=cv, op=OP.add)
                        if nxt is not None:
                            ptiles[d] = nxt

            # ---- final projection: outT = w_emb @ h2.T + b_emb ----
            eorder, lo = [], (nb - 1) // 2
            hi = lo + 1
            while lo >= 0 or hi < nb:
                if lo >= 0:
                    eorder.append(lo)
                if hi < nb:
                    eorder.append(hi)
                lo, hi = lo - 1, hi + 1
            for n in eorder:
                c0 = n * NCH
                pe = ippool.tile([NOUT, NCH], F32, name=f"pe{n}", tag="ip")
                for k in range(4):
                    nc.tensor.matmul(pe[:], sb_wemb[:, k, :],
                                     hb[DIRS[k // 2]][n][:, :, k % 2, :],
                                     start=(k == 0), stop=(k == 3))
                ob = work.tile([NOUT, NCH], F32, name=f"ob{n}", tag="ob", bufs=3)
                nc.scalar.activation(out=ob, in_=pe,
                                     func=AF.Identity, bias=sb_bemb[:, 0:1],
                                     scale=1.0)
                nc.sync.dma_start(out=outT[:, c0:c0 + NCH], in_=ob)

    nc.finalize()
    return nc


def _bf(a):
    return np.ascontiguousarray(a.astype(ml_dtypes.bfloat16))


def _f32(a):
    return np.ascontiguousarray(a.astype(np.float32))


def prep_shared(inputs, t_steps=T):
    """Host-side prep of the (core-independent) weight tensors.

    The z-gate rows (256:512) of w_ih, w_hh and the folded bias are negated
    so the device's sigmoid yields z' = 1 - z.
    """
    sh = {}
    for l in (0, 1):
        for d in DIRS:
            suf = f"l{l}{d}"
            w_ih = np.asarray(inputs[f"w_ih_{suf}"], np.float32).copy()
            w_hh = np.asarray(inputs[f"w_hh_{suf}"], np.float32).copy()
            b_ih = np.asarray(inputs[f"b_ih_{suf}"], np.float32)
            b_hh = np.asarray(inputs[f"b_hh_{suf}"], np.float32)
            w_ih[H:2 * H] *= -1.0
            w_hh[H:2 * H] *= -1.0
            kin = w_ih.shape[1] // 128
            sh[f"wih{l}{d}"] = _bf(w_ih.T.reshape(kin, 128, G3))
            sh[f"whh{l}{d}"] = _bf(w_hh.T.reshape(2, 128, G3))
            bg = b_ih.copy()
            bg[:2 * H] += b_hh[:2 * H]
            bg[H:2 * H] *= -1.0
            sh[f"bgi{l}{d}"] = _f32(bg.reshape(NM, 128).T)
            bhn_pc = b_hh[2 * H:].reshape(2, 128).T          # (128, 2)
            sh[f"bhn{l}{d}"] = _bf(
                np.broadcast_to(bhn_pc[:, :, None], (128, 2, BL)))
    w_emb = np.asarray(inputs["w_emb"], np.float32)                # (96, 512)
    sh["wemb"] = _bf(w_emb.T.reshape(4, 128, NOUT))
    sh["bemb"] = _f32(np.asarray(inputs["b_emb"], np.float32).reshape(NOUT, 1))
    sh["ident"] = _bf(np.eye(128, dtype=np.float32))
    return sh


def prep_in_maps(inputs, t_steps=T):
    x = np.asarray(inputs["x"], np.float32)[:t_steps]              # (T, B, NIN)
    sh = prep_shared(inputs, t_steps)
    tb = t_steps * BL
    in_maps = []
    for c in range(NCORES):
        xc = x[:, c * BL:(c + 1) * BL, :]                          # (T, BL, NIN)
        xT = xc.transpose(2, 0, 1).reshape(NIN, tb)                # (NIN, T*BL)
        m = dict(sh)
        m["xT"] = _bf(xT.reshape(2, 128, tb))
        in_maps.append(m)
    return in_maps


def assemble(results, t_steps=T):
    outs = []
    for c in range(NCORES):
        o = np.asarray(results[c]["outT"], np.float32)             # (96, T*BL)
        outs.append(o.reshape(NOUT, t_steps, BL).transpose(1, 2, 0))
    return np.concatenate(outs, axis=1)                            # (T, B, 96)


@functools.lru_cache(maxsize=2)
def get_nc(t_steps=T):
    return build_bass(t_steps)


_NEFF_CACHE = "/tmp/neff_cache_gru"


def _install_neff_cache():
    """Cache walrus-compiled NEFFs keyed by BIR content hash."""
    import hashlib
    import os
    import shutil
    import concourse.bass2jax as b2j
    if getattr(b2j, "_neff_cache_installed", False):
        return
    os.makedirs(_NEFF_CACHE, exist_ok=True)
    orig = b2j.compile_bir_kernel

    def cached(ant_bir_str, compile_dir_path, neff_name="file.neff", **kw):
        h = hashlib.sha256(ant_bir_str).hexdigest()[:24]
        cpath = os.path.join(_NEFF_CACHE, f"{h}.neff")
        dst = os.path.join(compile_dir_path, neff_name)
        if os.path.exists(cpath):
            shutil.copyfile(cpath, dst)
            return dst
        neff = orig(ant_bir_str, compile_dir_path, neff_name=neff_name, **kw)
        try:
            shutil.copyfile(neff, cpath)
        except OSError:
            pass
        return neff

    b2j.compile_bir_kernel = cached
    b2j._neff_cache_installed = True


def _install_ntff_hook():
    """Wire up the axon NTFF profile hook that this image's antenv lacks."""
    import types
    if "antenv.axon_hooks" not in sys.modules:
        mod = types.ModuleType("antenv.axon_hooks")
        holder = {}
        mod.set_axon_ntff_profile_hook = lambda h: holder.__setitem__("h", h)
        mod.get_axon_ntff_profile_hook = lambda: holder.get("h")
        sys.modules["antenv.axon_hooks"] = mod
        import antenv
        antenv.axon_hooks = mod
    else:
        mod = sys.modules["antenv.axon_hooks"]
    if mod.get_axon_ntff_profile_hook() is None:
        if "/root/.axon_site" not in sys.path:
            sys.path.insert(0, "/root/.axon_site")
        from trn_agent_boot.trn_boot import _ntff_profile_via_ctypes
        mod.set_axon_ntff_profile_hook(
            _ntff_profile_via_ctypes("/opt/axon/libaxon_pjrt.so"))
    import concourse.bass_utils as bu
    bu.upload_artifacts = lambda tmpdir: f"local:{tmpdir}"


def _run(inputs, t_steps=T, trace=False):
    from concourse.bass_utils import run_bass_kernel_spmd
    _install_neff_cache()
    if trace:
        _install_ntff_hook()
    nc = get_nc(t_steps)
    in_maps = prep_in_maps(inputs, t_steps)
    res = run_bass_kernel_spmd(nc, in_maps, list(range(NCORES)), trace=trace)
    return assemble(res.results, t_steps), res


def kernel(**inputs):
    out, _ = _run(inputs, T, trace=False)
    return out


def run_traced(inputs, t_steps=T):
    out, res = _run(inputs, t_steps, trace=True)
    trace_path = None
    if res.instructions_and_trace is not None:
        trace_path = res.instructions_and_trace[1]
    return out, res.exec_time_ns, trace_path
